# revision 20
# baseline (speedup 1.0000x reference)
"""Trainium2 Bass kernel for nn_ConcatHeadModule (pairwise concat-head scorer).

Math (reference):
    xc   = x.reshape(T, 2L)
    actH = tanh(xc @ W1H + cbH);  actM = tanh(xc @ W1M + cbM)
    AH   = actH @ L2H;            AM   = actM @ L2M
    scores[i,j] = sum_h wOut[h]*tanh(AH[i,h] + AM[j,h] + h2b[h]) + outBias

Sharding: row-shard the [T,T] score grid across 8 cores (96 rows each).
Each core builds the full AM^T (h-major) once, then for each of its 96
rows i evaluates tanh(AM^T[h, j] + (AH[i,h]+h2b[h])) with the pairwise
add fused into ScalarE's per-partition activation bias, and contracts
over h with TensorE (bf16) into a [1, 768] PSUM row.

All shapes are hardcoded (T=768, 2L=512, HID=512, HID2=512, 8 cores).
"""

import os
import sys

for _p in ("/root/.axon_site", "/root/.axon_site/_ro/trn_rl_repo", "/opt/trn_rl_repo"):
    if os.path.isdir(_p) and _p not in sys.path:
        sys.path.append(_p)

import ml_dtypes
import numpy as np

import concourse.bass as bass
import concourse.mybir as mybir
import concourse.tile as tile
from concourse import bacc
from concourse.bass_utils import run_bass_kernel_spmd

F32 = mybir.dt.float32
F32R = mybir.dt.float32r
BF16 = mybir.dt.bfloat16
TANH = mybir.ActivationFunctionType.Tanh

T = 768          # tokens
C = 512          # 2 * LDIMS (concat lstm state)
H = 512          # hidden1
H2 = 512         # hidden2
NCORES = 8
R = T // NCORES  # score rows per core
P = 128          # partitions
NKC = C // P     # contraction chunks over C
NKH = H // P     # chunks over H
NKH2 = H2 // P   # chunks over H2


def build_nc(rows: int = R) -> bass.Bass:
    nc = bacc.Bacc("TRN2", target_bir_lowering=False, num_devices=NCORES)

    xT = nc.dram_tensor("xT", [C, T], F32R, kind="ExternalInput")
    xTi = nc.dram_tensor("xTi", [C, rows], F32, kind="ExternalInput")
    w1h = nc.dram_tensor("w1h", [C, H], F32, kind="ExternalInput")
    w1m = nc.dram_tensor("w1m", [C, H], F32R, kind="ExternalInput")
    hid2h = nc.dram_tensor("hid2h", [H, H2], F32, kind="ExternalInput")
    hid2m = nc.dram_tensor("hid2m", [H, H2], F32R, kind="ExternalInput")
    cbh = nc.dram_tensor("cbh", [NKH, P, 1], F32, kind="ExternalInput")
    cbm = nc.dram_tensor("cbm", [NKH, P, 1], F32, kind="ExternalInput")
    h2b = nc.dram_tensor("h2b", [NKH2, P, 1], F32, kind="ExternalInput")
    wout = nc.dram_tensor("wout", [NKH2, P, 32], BF16, kind="ExternalInput")
    ob = nc.dram_tensor("ob", [P, 1], F32, kind="ExternalInput")
    out_rows = nc.dram_tensor("out_rows", [rows, T], F32, kind="ExternalOutput")

    with tile.TileContext(nc) as tc:
        _emit(tc, locals(), rows)
    nc.compile()
    return nc


def _emit(tc: tile.TileContext, io, rows: int):
    nc = tc.nc
    xT, xTi, w1h, w1m = io["xT"], io["xTi"], io["w1h"], io["w1m"]
    hid2h, hid2m = io["hid2h"], io["hid2m"]
    cbh, cbm, h2b, wout, ob = io["cbh"], io["cbm"], io["h2b"], io["wout"], io["ob"]
    out_rows = io["out_rows"]

    with tc.tile_pool(name="const", bufs=1) as const:
        setup_pool_cm = tc.tile_pool(name="setup_sb", bufs=1)
        setup = setup_pool_cm.__enter__()
        # ---- load inputs ----
        # DMA order = consumption order: tiny bias tiles first, then the
        # M-side chain that gates the first activations, then the H side.
        xT_sb = []
        xTi_sb = []
        w1h_sb = []
        w1m_sb = []
        l2h_sb = []
        l2m_sb = []
        cbh_sb = []
        cbm_sb = []
        h2b_sb = []
        wout_sb = []
        for k in range(NKH):
            t = setup.tile([P, 1], F32, name=f"cbm_sb{k}")
            nc.sync.dma_start(t[:], cbm[k, :, :])
            cbm_sb.append(t)
            t = setup.tile([P, 1], F32, name=f"cbh_sb{k}")
            nc.sync.dma_start(t[:], cbh[k, :, :])
            cbh_sb.append(t)
        for k in range(NKH2):
            t = setup.tile([P, 1], F32, name=f"h2b_sb{k}")
            nc.sync.dma_start(t[:], h2b[k, :, :])
            h2b_sb.append(t)
            t = const.tile([P, 32], BF16, name=f"wout_sb{k}")
            nc.sync.dma_start(t[:], wout[k, :, :])
            wout_sb.append(t)
        for k in range(NKC):
            t = setup.tile([P, H], F32R, name=f"w1m_sb{k}")
            nc.sync.dma_start(t[:], w1m[k * P:(k + 1) * P, :])
            w1m_sb.append(t)
            t = setup.tile([P, T], F32R, name=f"xT_sb{k}")
            nc.sync.dma_start(t[:], xT[k * P:(k + 1) * P, :])
            xT_sb.append(t)
        for k in range(NKH):
            t = setup.tile([P, H2], F32R, name=f"l2m_sb{k}")
            nc.sync.dma_start(t[:], hid2m[k * P:(k + 1) * P, :])
            l2m_sb.append(t)
        for k in range(NKC):
            t = setup.tile([P, rows], F32, name=f"xTi_sb{k}")
            nc.sync.dma_start(t[:], xTi[k * P:(k + 1) * P, :])
            xTi_sb.append(t)
            t = setup.tile([P, H], F32, name=f"w1h_sb{k}")
            nc.sync.dma_start(t[:], w1h[k * P:(k + 1) * P, :])
            w1h_sb.append(t)
        for k in range(NKH):
            t = setup.tile([P, H2], F32, name=f"l2h_sb{k}")
            nc.sync.dma_start(t[:], hid2h[k * P:(k + 1) * P, :])
            l2h_sb.append(t)
        ob_sb = const.tile([P, 1], F32, name="ob_sb")
        nc.sync.dma_start(ob_sb[:], ob[:, :])

        # ---- setup: actM^T = tanh(Mf^T + cbm), Mf^T = W1M^T @ xc^T ----
        NHALF = T // 2  # 384 <= 512 fp32 moving-operand limit
        setup_ps_cm = tc.tile_pool(name="setup_ps", bufs=2, space="PSUM")
        setup_ps = setup_ps_cm.__enter__()
        actMT = []
        for hc in range(NKH):
            amt = setup.tile([P, T], F32R, name=f"actMT{hc}")
            for n0 in range(0, T, NHALF):
                ps = setup_ps.tile([P, NHALF], F32, tag="setup")
                for cc in range(NKC):
                    nc.tensor.matmul(
                        ps[:],
                        lhsT=w1m_sb[cc][:, hc * P:(hc + 1) * P],
                        rhs=xT_sb[cc][:, n0:n0 + NHALF],
                        start=(cc == 0),
                        stop=(cc == NKC - 1),
                    )
                nc.scalar.activation(
                    amt[:, n0:n0 + NHALF], ps[:], TANH, bias=cbm_sb[hc][:]
                )
            actMT.append(amt)

        # actH^T for this core's rows = tanh(Hf^T + cbh)
        actHT = []
        for hc in range(NKH):
            aht = setup.tile([P, rows], F32, name=f"actHT{hc}")
            ps = setup_ps.tile([P, rows], F32, tag="setup")
            for cc in range(NKC):
                nc.tensor.matmul(
                    ps[:],
                    lhsT=w1h_sb[cc][:, hc * P:(hc + 1) * P],
                    rhs=xTi_sb[cc][:],
                    start=(cc == 0),
                    stop=(cc == NKC - 1),
                )
            nc.scalar.activation(aht[:], ps[:], TANH, bias=cbh_sb[hc][:])
            actHT.append(aht)

        # AM^T = L2M^T @ actM^T  (stored bf16 so the DVE pairwise add runs 4x)
        AMT = []
        for hc in range(NKH2):
            am = const.tile([P, T], BF16, name=f"AMT{hc}")
            for n0 in range(0, T, NHALF):
                ps = setup_ps.tile([P, NHALF], F32, tag="setup")
                for kc in range(NKH):
                    nc.tensor.matmul(
                        ps[:],
                        lhsT=l2m_sb[kc][:, hc * P:(hc + 1) * P],
                        rhs=actMT[kc][:, n0:n0 + NHALF],
                        start=(kc == 0),
                        stop=(kc == NKH - 1),
                    )
                nc.vector.tensor_copy(am[:, n0:n0 + NHALF], ps[:])
            AMT.append(am)

        # ABIAS[h, i] = AH^T[h, i] + h2b[h]
        ABIAS = []
        for hc in range(NKH2):
            ab = const.tile([P, rows], F32, name=f"ABIAS{hc}")
            ps = setup_ps.tile([P, rows], F32, tag="setup")
            for kc in range(NKH):
                nc.tensor.matmul(
                    ps[:],
                    lhsT=l2h_sb[kc][:, hc * P:(hc + 1) * P],
                    rhs=actHT[kc][:],
                    start=(kc == 0),
                    stop=(kc == NKH - 1),
                )
            nc.vector.tensor_scalar_add(ab[:], ps[:], h2b_sb[hc][:])
            ABIAS.append(ab)

        setup_ps_cm.__exit__(None, None, None)
        setup_pool_cm.__exit__(None, None, None)

        # ---- main loop ----
        # Per group of G rows: DVE builds bf16 pairwise-sum tiles (4x mode),
        # one big-FD tanh per h-chunk on ScalarE, TensorE contracts against
        # wOut with 4 rows per PSUM tile via tile_position col groups.
        if rows % 8 == 0:
            group_sizes = [8] * (rows // 8)
        elif rows % 4 == 0:
            group_sizes = [4] * (rows // 4)
        else:
            group_sizes = [1] * rows
        with (
            tc.tile_pool(name="spool", bufs=3) as spool,
            tc.tile_pool(name="zpool", bufs=NKH2 + 5) as zpool,
            tc.tile_pool(name="evbuf", bufs=3) as evpool,
            tc.tile_pool(name="row_ps", bufs=4, space="PSUM") as row_ps,
        ):
            i0 = 0
            for g, G in enumerate(group_sizes):
                QR = 4 if G % 4 == 0 else 1
                NQ = G // QR
                Zs = []
                for hc in range(NKH2):
                    S = spool.tile([P, G * T], BF16, tag="s", name=f"S{g}_{hc}")
                    for u in range(G):
                        nc.vector.tensor_scalar_add(
                            S[:, u * T:(u + 1) * T], AMT[hc][:],
                            ABIAS[hc][:, i0 + u:i0 + u + 1],
                        )
                    Z = zpool.tile([P, G * T], BF16, tag="z", name=f"Z{g}_{hc}")
                    nc.scalar.activation(Z[:], S[:], TANH)
                    Zs.append(Z)
                # Wave-scheduled contraction: wave w covers regions
                # (q, cg=(q+w)%QR) — pending PSUM accumulation groups sit in
                # disjoint banks (different q tiles), consecutive matmuls
                # rotate PE col strips so LDWEIGHTS overlaps in-flight
                # matmuls, and the stationary (wout[hc]) is constant across
                # each wave.
                psrs = [
                    row_ps.tile([P, T], F32, tag="row", name=f"psr{g}_{q}")
                    for q in range(NQ)
                ]
                for w in range(QR):
                    for hc in range(NKH2):
                        for n0, nw in ((0, 512), (512, T - 512)):
                            for q in range(NQ):
                                cg = (q + w) % QR
                                u = q * QR + cg
                                nc.tensor.matmul(
                                    psrs[q][32 * cg:32 * cg + 32, n0:n0 + nw],
                                    lhsT=wout_sb[hc][:],
                                    rhs=Zs[hc][:, u * T + n0:u * T + n0 + nw],
                                    start=(hc == 0),
                                    stop=(hc == NKH2 - 1),
                                    tile_position=(0, 32 * cg),
                                )
                for q in range(NQ):
                    ev = evpool.tile([P, T], F32, tag="ev", name=f"ev{g}_{q}")
                    nc.vector.tensor_scalar_add(ev[:], psrs[q][:], ob_sb[:])
                    for cg in range(QR):
                        i = i0 + q * QR + cg
                        nc.sync.dma_start(
                            out_rows[i:i + 1, :], ev[32 * cg:32 * cg + 1, :]
                        )
                i0 += G


def _prep_inputs(x, hidLayerFOH, hidLayerFOM, catBias, hid2Layer, hid2Bias,
                 outLayer, outBias, rows=R, ncores=NCORES):
    """Host-side layout prep (reshape/transpose/slice/cast only)."""
    x = np.asarray(x, np.float32)
    xc = x.reshape(T, C)
    xT_np = np.ascontiguousarray(xc.T)
    common = {
        "xT": xT_np,
        "w1h": np.ascontiguousarray(np.asarray(hidLayerFOH, np.float32)),
        "w1m": np.ascontiguousarray(np.asarray(hidLayerFOM, np.float32)),
        "hid2h": np.ascontiguousarray(np.asarray(hid2Layer, np.float32)[:H]),
        "hid2m": np.ascontiguousarray(np.asarray(hid2Layer, np.float32)[H:]),
        "cbh": np.ascontiguousarray(
            np.asarray(catBias[:H], np.float32).reshape(NKH, P, 1)),
        "cbm": np.ascontiguousarray(
            np.asarray(catBias[H:], np.float32).reshape(NKH, P, 1)),
        "h2b": np.ascontiguousarray(
            np.asarray(hid2Bias, np.float32).reshape(NKH2, P, 1)),
        "wout": np.ascontiguousarray(np.repeat(
            np.asarray(outLayer, np.float32).astype(ml_dtypes.bfloat16)
            .reshape(NKH2, P, 1), 32, axis=2)),
        "ob": np.full((P, 1), np.asarray(outBias, np.float32).reshape(()),
                      np.float32),
    }
    in_maps = []
    for c in range(ncores):
        m = dict(common)
        m["xTi"] = np.ascontiguousarray(xc[c * rows:(c + 1) * rows].T)
        in_maps.append(m)
    return in_maps


def kernel(x, hidLayerFOH, hidLayerFOM, catBias, hid2Layer, hid2Bias,
           outLayer, outBias, _trace=False):
    in_maps = _prep_inputs(x, hidLayerFOH, hidLayerFOM, catBias,
                           hid2Layer, hid2Bias, outLayer, outBias)
    nc = build_nc(R)
    res = run_bass_kernel_spmd(nc, in_maps, core_ids=list(range(NCORES)),
                               trace=_trace)
    out = np.concatenate([res.results[c]["out_rows"] for c in range(NCORES)], 0)
    if _trace:
        kernel.last_results = res
    return out.astype(np.float32)


# revision 24
# speedup vs baseline: 1.0262x; 1.0262x over previous
"""Trainium2 Bass kernel for nn_ConcatHeadModule (pairwise concat-head scorer).

Math (reference):
    xc   = x.reshape(T, 2L)
    actH = tanh(xc @ W1H + cbH);  actM = tanh(xc @ W1M + cbM)
    AH   = actH @ L2H;            AM   = actM @ L2M
    scores[i,j] = sum_h wOut[h]*tanh(AH[i,h] + AM[j,h] + h2b[h]) + outBias

Sharding: row-shard the [T,T] score grid across 8 cores (96 rows each).
Each core builds the full AM^T (h-major) once, then for each of its 96
rows i evaluates tanh(AM^T[h, j] + (AH[i,h]+h2b[h])) with the pairwise
add fused into ScalarE's per-partition activation bias, and contracts
over h with TensorE (bf16) into a [1, 768] PSUM row.

All shapes are hardcoded (T=768, 2L=512, HID=512, HID2=512, 8 cores).
"""

import os
import sys

for _p in ("/root/.axon_site", "/root/.axon_site/_ro/trn_rl_repo", "/opt/trn_rl_repo"):
    if os.path.isdir(_p) and _p not in sys.path:
        sys.path.append(_p)

import ml_dtypes
import numpy as np

import concourse.bass as bass
import concourse.mybir as mybir
import concourse.tile as tile
from concourse import bacc
from concourse.bass_utils import run_bass_kernel_spmd

F32 = mybir.dt.float32
F32R = mybir.dt.float32r
BF16 = mybir.dt.bfloat16
TANH = mybir.ActivationFunctionType.Tanh

T = 768          # tokens
C = 512          # 2 * LDIMS (concat lstm state)
H = 512          # hidden1
H2 = 512         # hidden2
NCORES = 8
R = T // NCORES  # score rows per core
P = 128          # partitions
NKC = C // P     # contraction chunks over C
NKH = H // P     # chunks over H
NKH2 = H2 // P   # chunks over H2


def build_nc(rows: int = R) -> bass.Bass:
    nc = bacc.Bacc("TRN2", target_bir_lowering=False, num_devices=NCORES)

    xT = nc.dram_tensor("xT", [C, T], F32R, kind="ExternalInput")
    xTi = nc.dram_tensor("xTi", [C, rows], F32, kind="ExternalInput")
    w1h = nc.dram_tensor("w1h", [C, H], F32, kind="ExternalInput")
    w1m = nc.dram_tensor("w1m", [C, H], F32R, kind="ExternalInput")
    hid2h = nc.dram_tensor("hid2h", [H, H2], F32, kind="ExternalInput")
    hid2m = nc.dram_tensor("hid2m", [H, H2], F32R, kind="ExternalInput")
    cbh = nc.dram_tensor("cbh", [P, NKH], F32, kind="ExternalInput")
    cbm = nc.dram_tensor("cbm", [P, NKH], F32, kind="ExternalInput")
    h2b = nc.dram_tensor("h2b", [P, NKH2], F32, kind="ExternalInput")
    wout = nc.dram_tensor("wout", [P, NKH2 * 32], BF16, kind="ExternalInput")
    ob = nc.dram_tensor("ob", [P, 1], F32, kind="ExternalInput")
    out_rows = nc.dram_tensor("out_rows", [rows, T], F32, kind="ExternalOutput")

    with tile.TileContext(nc) as tc:
        _emit(tc, locals(), rows)
    nc.compile()
    return nc


def _emit(tc: tile.TileContext, io, rows: int):
    nc = tc.nc
    xT, xTi, w1h, w1m = io["xT"], io["xTi"], io["w1h"], io["w1m"]
    hid2h, hid2m = io["hid2h"], io["hid2m"]
    cbh, cbm, h2b, wout, ob = io["cbh"], io["cbm"], io["h2b"], io["wout"], io["ob"]
    out_rows = io["out_rows"]

    with tc.tile_pool(name="const", bufs=1) as const:
        setup_pool_cm = tc.tile_pool(name="setup_sb", bufs=1)
        setup = setup_pool_cm.__enter__()
        # ---- load inputs ----
        # One coalesced DMA per tensor (SP-queue issue is ~0.65us per DMA,
        # so 8 big DMAs beat ~40 small ones), ordered by first use: biases,
        # then the M-side chain that gates the first activations, then the
        # H side.
        def load_coalesced(pool, name, dram, k, inner, dt):
            t = pool.tile([P, k * inner], dt, name=name)
            nc.sync.dma_start(
                t[:].rearrange("p (k t) -> p k t", k=k),
                dram[:].rearrange("(k p) t -> p k t", p=P),
            )
            return t, [t[:, j * inner:(j + 1) * inner] for j in range(k)]

        cb_all = const.tile([P, 2 * NKH + NKH2 + 1], F32, name="cb_all")
        nc.sync.dma_start(cb_all[:, 0:NKH], cbm[:, :])
        nc.sync.dma_start(cb_all[:, NKH:2 * NKH], cbh[:, :])
        nc.sync.dma_start(cb_all[:, 2 * NKH:2 * NKH + NKH2], h2b[:, :])
        nc.sync.dma_start(cb_all[:, 2 * NKH + NKH2:], ob[:, :])
        cbm_sb = [cb_all[:, k:k + 1] for k in range(NKH)]
        cbh_sb = [cb_all[:, NKH + k:NKH + k + 1] for k in range(NKH)]
        h2b_sb = [cb_all[:, 2 * NKH + k:2 * NKH + k + 1] for k in range(NKH2)]
        ob_sb = cb_all[:, 2 * NKH + NKH2:2 * NKH + NKH2 + 1]
        wout_all = const.tile([P, NKH2 * 32], BF16, name="wout_all")
        nc.sync.dma_start(wout_all[:], wout[:, :])
        wout_sb = [wout_all[:, 32 * k:32 * (k + 1)] for k in range(NKH2)]

        _, w1m_sb = load_coalesced(setup, "w1m_all", w1m, NKC, H, F32R)
        _, xT_sb = load_coalesced(setup, "xT_all", xT, NKC, T, F32R)
        _, l2m_sb = load_coalesced(setup, "l2m_all", hid2m, NKH, H2, F32R)
        _, xTi_sb = load_coalesced(setup, "xTi_all", xTi, NKC, rows, F32)
        _, w1h_sb = load_coalesced(setup, "w1h_all", w1h, NKC, H, F32)
        _, l2h_sb = load_coalesced(setup, "l2h_all", hid2h, NKH, H2, F32)

        # ---- setup: actM^T = tanh(Mf^T + cbm), Mf^T = W1M^T @ xc^T ----
        NHALF = T // 2  # 384 <= 512 fp32 moving-operand limit
        setup_ps_cm = tc.tile_pool(name="setup_ps", bufs=2, space="PSUM")
        setup_ps = setup_ps_cm.__enter__()
        actMT = []
        for hc in range(NKH):
            amt = setup.tile([P, T], F32R, name=f"actMT{hc}")
            for n0 in range(0, T, NHALF):
                ps = setup_ps.tile([P, NHALF], F32, tag="setup")
                for cc in range(NKC):
                    nc.tensor.matmul(
                        ps[:],
                        lhsT=w1m_sb[cc][:, hc * P:(hc + 1) * P],
                        rhs=xT_sb[cc][:, n0:n0 + NHALF],
                        start=(cc == 0),
                        stop=(cc == NKC - 1),
                    )
                nc.scalar.activation(
                    amt[:, n0:n0 + NHALF], ps[:], TANH, bias=cbm_sb[hc][:]
                )
            actMT.append(amt)

        # actH^T for this core's rows = tanh(Hf^T + cbh)
        actHT = []
        for hc in range(NKH):
            aht = setup.tile([P, rows], F32, name=f"actHT{hc}")
            ps = setup_ps.tile([P, rows], F32, tag="setup")
            for cc in range(NKC):
                nc.tensor.matmul(
                    ps[:],
                    lhsT=w1h_sb[cc][:, hc * P:(hc + 1) * P],
                    rhs=xTi_sb[cc][:],
                    start=(cc == 0),
                    stop=(cc == NKC - 1),
                )
            nc.scalar.activation(aht[:], ps[:], TANH, bias=cbh_sb[hc][:])
            actHT.append(aht)

        # AM^T = L2M^T @ actM^T  (stored bf16 so the DVE pairwise add runs 4x)
        AMT = []
        for hc in range(NKH2):
            am = const.tile([P, T], BF16, name=f"AMT{hc}")
            for n0 in range(0, T, NHALF):
                ps = setup_ps.tile([P, NHALF], F32, tag="setup")
                for kc in range(NKH):
                    nc.tensor.matmul(
                        ps[:],
                        lhsT=l2m_sb[kc][:, hc * P:(hc + 1) * P],
                        rhs=actMT[kc][:, n0:n0 + NHALF],
                        start=(kc == 0),
                        stop=(kc == NKH - 1),
                    )
                nc.vector.tensor_copy(am[:, n0:n0 + NHALF], ps[:])
            AMT.append(am)

        # ABIAS[h, i] = AH^T[h, i] + h2b[h]
        ABIAS = []
        for hc in range(NKH2):
            ab = const.tile([P, rows], F32, name=f"ABIAS{hc}")
            ps = setup_ps.tile([P, rows], F32, tag="setup")
            for kc in range(NKH):
                nc.tensor.matmul(
                    ps[:],
                    lhsT=l2h_sb[kc][:, hc * P:(hc + 1) * P],
                    rhs=actHT[kc][:],
                    start=(kc == 0),
                    stop=(kc == NKH - 1),
                )
            nc.vector.tensor_scalar_add(ab[:], ps[:], h2b_sb[hc][:])
            ABIAS.append(ab)

        setup_ps_cm.__exit__(None, None, None)
        setup_pool_cm.__exit__(None, None, None)

        # ---- main loop ----
        # Per group of G rows: DVE builds bf16 pairwise-sum tiles (4x mode),
        # one big-FD tanh per h-chunk on ScalarE, TensorE contracts against
        # wOut with 4 rows per PSUM tile via tile_position col groups.
        if rows % 8 == 0:
            group_sizes = [8] * (rows // 8)
        elif rows % 4 == 0:
            group_sizes = [4] * (rows // 4)
        else:
            group_sizes = [1] * rows
        with (
            tc.tile_pool(name="spool", bufs=3) as spool,
            tc.tile_pool(name="zpool", bufs=NKH2 + 5) as zpool,
            tc.tile_pool(name="evbuf", bufs=3) as evpool,
            tc.tile_pool(name="row_ps", bufs=4, space="PSUM") as row_ps,
        ):
            i0 = 0
            for g, G in enumerate(group_sizes):
                QR = 4 if G % 4 == 0 else 1
                NQ = G // QR
                Zs = []
                for hc in range(NKH2):
                    S = spool.tile([P, G * T], BF16, tag="s", name=f"S{g}_{hc}")
                    for u in range(G):
                        nc.vector.tensor_scalar_add(
                            S[:, u * T:(u + 1) * T], AMT[hc][:],
                            ABIAS[hc][:, i0 + u:i0 + u + 1],
                        )
                    Z = zpool.tile([P, G * T], BF16, tag="z", name=f"Z{g}_{hc}")
                    nc.scalar.activation(Z[:], S[:], TANH)
                    Zs.append(Z)
                # Wave-scheduled contraction: wave w covers regions
                # (q, cg=(q+w)%QR) — pending PSUM accumulation groups sit in
                # disjoint banks (different q tiles), consecutive matmuls
                # rotate PE col strips so LDWEIGHTS overlaps in-flight
                # matmuls, and the stationary (wout[hc]) is constant across
                # each wave.
                psrs = [
                    row_ps.tile([P, T], F32, tag="row", name=f"psr{g}_{q}")
                    for q in range(NQ)
                ]
                for w in range(QR):
                    for hc in range(NKH2):
                        for n0, nw in ((0, 512), (512, T - 512)):
                            for q in range(NQ):
                                cg = (q + w) % QR
                                u = q * QR + cg
                                nc.tensor.matmul(
                                    psrs[q][32 * cg:32 * cg + 32, n0:n0 + nw],
                                    lhsT=wout_sb[hc][:],
                                    rhs=Zs[hc][:, u * T + n0:u * T + n0 + nw],
                                    start=(hc == 0),
                                    stop=(hc == NKH2 - 1),
                                    tile_position=(0, 32 * cg),
                                )
                for q in range(NQ):
                    ev = evpool.tile([P, T], F32, tag="ev", name=f"ev{g}_{q}")
                    nc.vector.tensor_scalar_add(ev[:], psrs[q][:], ob_sb[:])
                    for cg in range(QR):
                        i = i0 + q * QR + cg
                        nc.sync.dma_start(
                            out_rows[i:i + 1, :], ev[32 * cg:32 * cg + 1, :]
                        )
                i0 += G


def _prep_inputs(x, hidLayerFOH, hidLayerFOM, catBias, hid2Layer, hid2Bias,
                 outLayer, outBias, rows=R, ncores=NCORES):
    """Host-side layout prep (reshape/transpose/slice/cast only)."""
    x = np.asarray(x, np.float32)
    xc = x.reshape(T, C)
    xT_np = np.ascontiguousarray(xc.T)
    common = {
        "xT": xT_np,
        "w1h": np.ascontiguousarray(np.asarray(hidLayerFOH, np.float32)),
        "w1m": np.ascontiguousarray(np.asarray(hidLayerFOM, np.float32)),
        "hid2h": np.ascontiguousarray(np.asarray(hid2Layer, np.float32)[:H]),
        "hid2m": np.ascontiguousarray(np.asarray(hid2Layer, np.float32)[H:]),
        "cbh": np.ascontiguousarray(
            np.asarray(catBias[:H], np.float32).reshape(NKH, P).T),
        "cbm": np.ascontiguousarray(
            np.asarray(catBias[H:], np.float32).reshape(NKH, P).T),
        "h2b": np.ascontiguousarray(
            np.asarray(hid2Bias, np.float32).reshape(NKH2, P).T),
        "wout": np.ascontiguousarray(np.repeat(
            np.asarray(outLayer, np.float32).astype(ml_dtypes.bfloat16)
            .reshape(NKH2, P).T, 32, axis=1)),
        "ob": np.full((P, 1), np.asarray(outBias, np.float32).reshape(()),
                      np.float32),
    }
    in_maps = []
    for c in range(ncores):
        m = dict(common)
        m["xTi"] = np.ascontiguousarray(xc[c * rows:(c + 1) * rows].T)
        in_maps.append(m)
    return in_maps


def kernel(x, hidLayerFOH, hidLayerFOM, catBias, hid2Layer, hid2Bias,
           outLayer, outBias, _trace=False):
    in_maps = _prep_inputs(x, hidLayerFOH, hidLayerFOM, catBias,
                           hid2Layer, hid2Bias, outLayer, outBias)
    nc = build_nc(R)
    res = run_bass_kernel_spmd(nc, in_maps, core_ids=list(range(NCORES)),
                               trace=_trace)
    out = np.concatenate([res.results[c]["out_rows"] for c in range(NCORES)], 0)
    if _trace:
        kernel.last_results = res
    return out.astype(np.float32)


# revision 26
# speedup vs baseline: 1.0585x; 1.0314x over previous
"""Trainium2 Bass kernel for nn_ConcatHeadModule (pairwise concat-head scorer).

Math (reference):
    xc   = x.reshape(T, 2L)
    actH = tanh(xc @ W1H + cbH);  actM = tanh(xc @ W1M + cbM)
    AH   = actH @ L2H;            AM   = actM @ L2M
    scores[i,j] = sum_h wOut[h]*tanh(AH[i,h] + AM[j,h] + h2b[h]) + outBias

Sharding: row-shard the [T,T] score grid across 8 cores (96 rows each).
Each core builds the full AM^T (h-major) once, then for each of its 96
rows i evaluates tanh(AM^T[h, j] + (AH[i,h]+h2b[h])) with the pairwise
add fused into ScalarE's per-partition activation bias, and contracts
over h with TensorE (bf16) into a [1, 768] PSUM row.

All shapes are hardcoded (T=768, 2L=512, HID=512, HID2=512, 8 cores).
"""

import os
import sys

for _p in ("/root/.axon_site", "/root/.axon_site/_ro/trn_rl_repo", "/opt/trn_rl_repo"):
    if os.path.isdir(_p) and _p not in sys.path:
        sys.path.append(_p)

import ml_dtypes
import numpy as np

import concourse.bass as bass
import concourse.mybir as mybir
import concourse.tile as tile
from concourse import bacc
from concourse.bass_utils import run_bass_kernel_spmd

F32 = mybir.dt.float32
F32R = mybir.dt.float32r
BF16 = mybir.dt.bfloat16
TANH = mybir.ActivationFunctionType.Tanh

T = 768          # tokens
C = 512          # 2 * LDIMS (concat lstm state)
H = 512          # hidden1
H2 = 512         # hidden2
NCORES = 8
R = T // NCORES  # score rows per core
P = 128          # partitions
NKC = C // P     # contraction chunks over C
NKH = H // P     # chunks over H
NKH2 = H2 // P   # chunks over H2


def build_nc(rows: int = R) -> bass.Bass:
    nc = bacc.Bacc("TRN2", target_bir_lowering=False, num_devices=NCORES)

    xT = nc.dram_tensor("xT", [C, T], F32R, kind="ExternalInput")
    xTi = nc.dram_tensor("xTi", [C, rows], F32, kind="ExternalInput")
    w1h = nc.dram_tensor("w1h", [C, H], F32, kind="ExternalInput")
    w1m = nc.dram_tensor("w1m", [C, H], F32R, kind="ExternalInput")
    hid2h = nc.dram_tensor("hid2h", [H, H2], F32, kind="ExternalInput")
    hid2m = nc.dram_tensor("hid2m", [H, H2], F32R, kind="ExternalInput")
    cbh = nc.dram_tensor("cbh", [P, NKH], F32, kind="ExternalInput")
    cbm = nc.dram_tensor("cbm", [P, NKH], F32, kind="ExternalInput")
    h2b = nc.dram_tensor("h2b", [P, NKH2], F32, kind="ExternalInput")
    wout = nc.dram_tensor("wout", [P, NKH2 * 32], BF16, kind="ExternalInput")
    ob = nc.dram_tensor("ob", [P, 1], F32, kind="ExternalInput")
    out_rows = nc.dram_tensor("out_rows", [rows, T], F32, kind="ExternalOutput")

    with tile.TileContext(nc) as tc:
        _emit(tc, locals(), rows)
    nc.compile()
    return nc


def _emit(tc: tile.TileContext, io, rows: int):
    nc = tc.nc
    xT, xTi, w1h, w1m = io["xT"], io["xTi"], io["w1h"], io["w1m"]
    hid2h, hid2m = io["hid2h"], io["hid2m"]
    cbh, cbm, h2b, wout, ob = io["cbh"], io["cbm"], io["h2b"], io["wout"], io["ob"]
    out_rows = io["out_rows"]

    with tc.tile_pool(name="const", bufs=1) as const:
        setup_pool_cm = tc.tile_pool(name="setup_sb", bufs=1)
        setup = setup_pool_cm.__enter__()
        # ---- load inputs ----
        # One coalesced DMA per tensor (SP-queue issue is ~0.65us per DMA,
        # so 8 big DMAs beat ~40 small ones), ordered by first use: biases,
        # then the M-side chain that gates the first activations, then the
        # H side.
        def load_coalesced(pool, name, dram, k, inner, dt, eng=None):
            t = pool.tile([P, k * inner], dt, name=name)
            (eng or nc.sync).dma_start(
                t[:].rearrange("p (k t) -> p k t", k=k),
                dram[:].rearrange("(k p) t -> p k t", p=P),
            )
            return t, [t[:, j * inner:(j + 1) * inner] for j in range(k)]

        cb_all = const.tile([P, 2 * NKH + NKH2 + 1], F32, name="cb_all")
        nc.sync.dma_start(cb_all[:, 0:NKH], cbm[:, :])
        nc.sync.dma_start(cb_all[:, NKH:2 * NKH], cbh[:, :])
        nc.sync.dma_start(cb_all[:, 2 * NKH:2 * NKH + NKH2], h2b[:, :])
        nc.sync.dma_start(cb_all[:, 2 * NKH + NKH2:], ob[:, :])
        cbm_sb = [cb_all[:, k:k + 1] for k in range(NKH)]
        cbh_sb = [cb_all[:, NKH + k:NKH + k + 1] for k in range(NKH)]
        h2b_sb = [cb_all[:, 2 * NKH + k:2 * NKH + k + 1] for k in range(NKH2)]
        ob_sb = cb_all[:, 2 * NKH + NKH2:2 * NKH + NKH2 + 1]
        wout_all = const.tile([P, NKH2 * 32], BF16, name="wout_all")
        nc.sync.dma_start(wout_all[:], wout[:, :])
        wout_sb = [wout_all[:, 32 * k:32 * (k + 1)] for k in range(NKH2)]

        _, w1m_sb = load_coalesced(setup, "w1m_all", w1m, NKC, H, F32R,
                                   eng=nc.scalar)
        _, xT_sb = load_coalesced(setup, "xT_all", xT, NKC, T, F32R,
                                  eng=nc.scalar)
        _, l2m_sb = load_coalesced(setup, "l2m_all", hid2m, NKH, H2, F32R)
        _, xTi_sb = load_coalesced(setup, "xTi_all", xTi, NKC, rows, F32)
        _, w1h_sb = load_coalesced(setup, "w1h_all", w1h, NKC, H, F32)
        _, l2h_sb = load_coalesced(setup, "l2h_all", hid2h, NKH, H2, F32)

        # ---- setup: actM^T = tanh(Mf^T + cbm), Mf^T = W1M^T @ xc^T ----
        NHALF = T // 2  # 384 <= 512 fp32 moving-operand limit
        setup_ps_cm = tc.tile_pool(name="setup_ps", bufs=4, space="PSUM")
        setup_ps = setup_ps_cm.__enter__()
        actMT = []
        for hc in range(NKH):
            amt = setup.tile([P, T], F32R, name=f"actMT{hc}")
            for n0 in range(0, T, NHALF):
                ps = setup_ps.tile([P, NHALF], F32, tag="setup")
                for cc in range(NKC):
                    nc.tensor.matmul(
                        ps[:],
                        lhsT=w1m_sb[cc][:, hc * P:(hc + 1) * P],
                        rhs=xT_sb[cc][:, n0:n0 + NHALF],
                        start=(cc == 0),
                        stop=(cc == NKC - 1),
                    )
                nc.scalar.activation(
                    amt[:, n0:n0 + NHALF], ps[:], TANH, bias=cbm_sb[hc][:]
                )
            actMT.append(amt)

        # actH^T for this core's rows = tanh(Hf^T + cbh)
        actHT = []
        for hc in range(NKH):
            aht = setup.tile([P, rows], F32, name=f"actHT{hc}")
            ps = setup_ps.tile([P, rows], F32, tag="setup")
            for cc in range(NKC):
                nc.tensor.matmul(
                    ps[:],
                    lhsT=w1h_sb[cc][:, hc * P:(hc + 1) * P],
                    rhs=xTi_sb[cc][:],
                    start=(cc == 0),
                    stop=(cc == NKC - 1),
                )
            nc.scalar.activation(aht[:], ps[:], TANH, bias=cbh_sb[hc][:])
            actHT.append(aht)

        # AM^T = L2M^T @ actM^T  (stored bf16 so the DVE pairwise add runs 4x)
        AMT = []
        for hc in range(NKH2):
            am = const.tile([P, T], BF16, name=f"AMT{hc}")
            for n0 in range(0, T, NHALF):
                ps = setup_ps.tile([P, NHALF], F32, tag="setup")
                for kc in range(NKH):
                    nc.tensor.matmul(
                        ps[:],
                        lhsT=l2m_sb[kc][:, hc * P:(hc + 1) * P],
                        rhs=actMT[kc][:, n0:n0 + NHALF],
                        start=(kc == 0),
                        stop=(kc == NKH - 1),
                    )
                nc.vector.tensor_copy(am[:, n0:n0 + NHALF], ps[:])
            AMT.append(am)

        # ABIAS[h, i] = AH^T[h, i] + h2b[h]
        ABIAS = []
        for hc in range(NKH2):
            ab = const.tile([P, rows], F32, name=f"ABIAS{hc}")
            ps = setup_ps.tile([P, rows], F32, tag="setup")
            for kc in range(NKH):
                nc.tensor.matmul(
                    ps[:],
                    lhsT=l2h_sb[kc][:, hc * P:(hc + 1) * P],
                    rhs=actHT[kc][:],
                    start=(kc == 0),
                    stop=(kc == NKH - 1),
                )
            nc.vector.tensor_scalar_add(ab[:], ps[:], h2b_sb[hc][:])
            ABIAS.append(ab)

        setup_ps_cm.__exit__(None, None, None)
        setup_pool_cm.__exit__(None, None, None)

        # ---- main loop ----
        # Per group of G rows: DVE builds bf16 pairwise-sum tiles (4x mode),
        # one big-FD tanh per h-chunk on ScalarE, TensorE contracts against
        # wOut with 4 rows per PSUM tile via tile_position col groups.
        if rows % 8 == 0 and rows >= 16:
            group_sizes = [8] * (rows // 8 - 1) + [4, 4]
        elif rows % 4 == 0:
            group_sizes = [4] * (rows // 4)
        else:
            group_sizes = [1] * rows
        with (
            tc.tile_pool(name="spool", bufs=3) as spool,
            tc.tile_pool(name="zpool", bufs=NKH2 + 5) as zpool,
            tc.tile_pool(name="evbuf", bufs=3) as evpool,
            tc.tile_pool(name="row_ps", bufs=4, space="PSUM") as row_ps,
        ):
            i0 = 0
            for g, G in enumerate(group_sizes):
                QR = 4 if G % 8 == 0 else (2 if G % 4 == 0 else 1)
                NQ = G // QR
                Zs = []
                for hc in range(NKH2):
                    S = spool.tile([P, G * T], BF16, tag="s", name=f"S{g}_{hc}")
                    for u in range(G):
                        nc.vector.tensor_scalar_add(
                            S[:, u * T:(u + 1) * T], AMT[hc][:],
                            ABIAS[hc][:, i0 + u:i0 + u + 1],
                        )
                    Z = zpool.tile([P, G * T], BF16, tag="z", name=f"Z{g}_{hc}")
                    nc.scalar.activation(Z[:], S[:], TANH)
                    Zs.append(Z)
                # Wave-scheduled contraction: wave w covers regions
                # (q, cg=(q+w)%QR) — pending PSUM accumulation groups sit in
                # disjoint banks (different q tiles), consecutive matmuls
                # rotate PE col strips so LDWEIGHTS overlaps in-flight
                # matmuls, and the stationary (wout[hc]) is constant across
                # each wave.
                psrs = [
                    row_ps.tile([P, T], F32, tag="row", name=f"psr{g}_{q}")
                    for q in range(NQ)
                ]
                for w in range(QR):
                    for hc in range(NKH2):
                        for n0, nw in ((0, 512), (512, T - 512)):
                            for q in range(NQ):
                                cg = (q + w) % QR
                                u = q * QR + cg
                                nc.tensor.matmul(
                                    psrs[q][32 * cg:32 * cg + 32, n0:n0 + nw],
                                    lhsT=wout_sb[hc][:],
                                    rhs=Zs[hc][:, u * T + n0:u * T + n0 + nw],
                                    start=(hc == 0),
                                    stop=(hc == NKH2 - 1),
                                    tile_position=(0, 32 * cg),
                                )
                for q in range(NQ):
                    ev = evpool.tile([P, T], F32, tag="ev", name=f"ev{g}_{q}")
                    nc.vector.tensor_scalar_add(
                        ev[0:32 * QR, :], psrs[q][0:32 * QR, :],
                        ob_sb[0:32 * QR, :],
                    )
                    for cg in range(QR):
                        i = i0 + q * QR + cg
                        nc.sync.dma_start(
                            out_rows[i:i + 1, :], ev[32 * cg:32 * cg + 1, :]
                        )
                i0 += G


def _prep_inputs(x, hidLayerFOH, hidLayerFOM, catBias, hid2Layer, hid2Bias,
                 outLayer, outBias, rows=R, ncores=NCORES):
    """Host-side layout prep (reshape/transpose/slice/cast only)."""
    x = np.asarray(x, np.float32)
    xc = x.reshape(T, C)
    xT_np = np.ascontiguousarray(xc.T)
    common = {
        "xT": xT_np,
        "w1h": np.ascontiguousarray(np.asarray(hidLayerFOH, np.float32)),
        "w1m": np.ascontiguousarray(np.asarray(hidLayerFOM, np.float32)),
        "hid2h": np.ascontiguousarray(np.asarray(hid2Layer, np.float32)[:H]),
        "hid2m": np.ascontiguousarray(np.asarray(hid2Layer, np.float32)[H:]),
        "cbh": np.ascontiguousarray(
            np.asarray(catBias[:H], np.float32).reshape(NKH, P).T),
        "cbm": np.ascontiguousarray(
            np.asarray(catBias[H:], np.float32).reshape(NKH, P).T),
        "h2b": np.ascontiguousarray(
            np.asarray(hid2Bias, np.float32).reshape(NKH2, P).T),
        "wout": np.ascontiguousarray(np.repeat(
            np.asarray(outLayer, np.float32).astype(ml_dtypes.bfloat16)
            .reshape(NKH2, P).T, 32, axis=1)),
        "ob": np.full((P, 1), np.asarray(outBias, np.float32).reshape(()),
                      np.float32),
    }
    in_maps = []
    for c in range(ncores):
        m = dict(common)
        m["xTi"] = np.ascontiguousarray(xc[c * rows:(c + 1) * rows].T)
        in_maps.append(m)
    return in_maps


def kernel(x, hidLayerFOH, hidLayerFOM, catBias, hid2Layer, hid2Bias,
           outLayer, outBias, _trace=False):
    in_maps = _prep_inputs(x, hidLayerFOH, hidLayerFOM, catBias,
                           hid2Layer, hid2Bias, outLayer, outBias)
    nc = build_nc(R)
    res = run_bass_kernel_spmd(nc, in_maps, core_ids=list(range(NCORES)),
                               trace=_trace)
    out = np.concatenate([res.results[c]["out_rows"] for c in range(NCORES)], 0)
    if _trace:
        kernel.last_results = res
    return out.astype(np.float32)
